# revision 1
# baseline (speedup 1.0000x reference)
"""DeepFM (nn_DeepFM_77558519431939) Trainium2 Bass kernel.

Strategy (8 NeuronCores, SPMD, no collectives):
  - Replicate the embedding table on every core; data-parallel the batch
    (16384 samples -> 2048 per core).  Each gathered row is fetched exactly
    once across the fleet, and there is no all-to-all.
  - Host-side prep builds an augmented table [S, 12]: 10 embedding dims,
    w_first value (first-order weight) in col 10, zero pad in col 11.  One
    indirect-DMA gather per 4096 rows fetches embeddings AND first-order
    weights together.
  - Gathered rows land sample-on-partition; PE transposes flip them into a
    feature-major activation matrix X [104, 2048]:
        rows f*12+e (e<10): emb dim e of field f
        rows f*12+10:       w_first value of field f
        rows f*12+11:       zero pad
        rows 96..102:       raw dense features (transposed on host)
        row 103:            constant 1.0 (bias row)
  - The whole DeepFM head is then a handful of matmuls per 512-column tile
    with all the small weights folded on the host:
        H1 = relu(W1s^T X)            (dense-proj + b1 folded into W1s)
        H2 = relu(W2^T H1 + b2)
        SD = SDw^T X                  (rows 0..9 = s, 10..19 = dense_emb,
                                       row 20 = first-order linear term)
        XSQ = [X[0:96]^2 ; SD[0:20]^2 ; SD[20]]
        FIN = esq^T XSQ + W3^T H2     (esq = +-0.5 masks + lin passthrough)
        out = sigmoid(FIN)
"""

import os
from contextlib import ExitStack

import numpy as np

import concourse.bass as bass
import concourse.bacc as bacc
import concourse.mybir as mybir
import concourse.tile as tile

# ---- problem constants (hardcoded; must match the reference) ----
VOCABS = [1000000, 500000, 200000, 100000, 50000, 10000, 5000, 1000]
S = int(np.sum(VOCABS))  # 1,866,000
OFFSETS = np.concatenate([[0], np.cumsum(VOCABS)[:-1]]).astype(np.int64)
B = 16384
EMB = 10
N_DENSE = 7
F = len(VOCABS)  # 8
HID = 128

N_CORES = 8
BL = B // N_CORES  # 2048 per core
RW = 12            # augmented table row width (10 emb + wf + pad)
KX = 104           # X partition rows: 96 gathered + 7 dense + 1 const
NSQ = 117          # XSQ rows: 96 emb^2 + 10 s^2 + 10 demb^2 + 1 lin
NBLK = BL // 128   # 16 sample blocks of 128
NT = BL // 512     # 4 column tiles of 512
GCH = 4            # gather chunk count (4 blocks of samples each)

F32 = mybir.dt.float32
I32 = mybir.dt.int32

_cached = {}


def _build_program(debug_dump=False):
    """Build the SPMD Bass program (same for all cores)."""
    nc = bacc.Bacc("TRN2", target_bir_lowering=False, debug=False)

    tab_d = nc.dram_tensor("tab", [S, RW], F32, kind="ExternalInput").ap()
    idx_d = nc.dram_tensor("idxs", [128, 128], I32, kind="ExternalInput").ap()
    dn8_d = nc.dram_tensor("dn8", [8, BL], F32, kind="ExternalInput").ap()
    # all small weights packed into one tensor: one DMA, one sem wait
    # cols: idn 0:128 | w1s 128:256 | w2 256:384 | b2 384 | sdw 385:405 |
    #       a1 405 | esq 406 | es2 407 | w3 408
    wpk_d = nc.dram_tensor("wpk", [128, 409], F32, kind="ExternalInput").ap()
    out_d = nc.dram_tensor("out", [1, BL], F32, kind="ExternalOutput").ap()
    if debug_dump:
        xdmp_d = nc.dram_tensor("xdmp", [KX, BL], F32, kind="ExternalOutput").ap()
        fdmp_d = nc.dram_tensor("fdmp", [1, BL], F32, kind="ExternalOutput").ap()

    with ExitStack() as ctx:
        tc = ctx.enter_context(tile.TileContext(nc))
        const = ctx.enter_context(tc.tile_pool(name="const", bufs=1))
        gpool = ctx.enter_context(tc.tile_pool(name="gch", bufs=128))
        hpool = ctx.enter_context(tc.tile_pool(name="h", bufs=2))
        qpool = ctx.enter_context(tc.tile_pool(name="xsq", bufs=2))
        pp_x = ctx.enter_context(tc.tile_pool(name="ppx", bufs=2, space="PSUM"))
        pp_h = ctx.enter_context(tc.tile_pool(name="pph", bufs=2, space="PSUM"))
        pp_s = ctx.enter_context(tc.tile_pool(name="pps", bufs=2, space="PSUM"))
        pp_f = ctx.enter_context(tc.tile_pool(name="ppf", bufs=2, space="PSUM"))

        # index tile first: the gathers depend only on it
        idx_t = const.tile([128, 128], I32)
        nc.sync.dma_start(idx_t[:], idx_d[:])

        # constants: one packed tile, sliced below
        wpk_t = const.tile([128, 409], F32)
        nc.sync.dma_start(wpk_t[:], wpk_d[:])
        idn_t = wpk_t[:, 0:128]
        w1s_t = wpk_t[0:KX, 128:256]
        w2_t = wpk_t[:, 256:384]
        b2_t = wpk_t[:, 384:385]
        sdw_t = wpk_t[0:KX, 385:405]
        a1_t = wpk_t[0:KX, 405:406]
        esq_t = wpk_t[0:96, 406:407]
        es2_t = wpk_t[0:20, 407:408]
        w3_t = wpk_t[:, 408:409]

        # X: feature-major activations
        x_t = const.tile([KX, BL], F32)
        nc.sync.dma_start(x_t[96:104, :], dn8_d[:])

        out_sb = const.tile([1, BL], F32)
        if debug_dump:
            fin_sb = const.tile([1, BL], F32)

        RELU = mybir.ActivationFunctionType.Relu
        SQUARE = mybir.ActivationFunctionType.Square
        SIGMOID = mybir.ActivationFunctionType.Sigmoid

        for j in range(NBLK):
            cols = slice(128 * j, 128 * (j + 1))
            gb = gpool.tile([128, F, RW], F32, tag="gch")
            for f in range(F):
                # HW indirect DMA: one offset per partition per instruction
                nc.gpsimd.indirect_dma_start(
                    out=gb[:, f, :],
                    out_offset=None,
                    in_=tab_d[:],
                    in_offset=bass.IndirectOffsetOnAxis(
                        ap=idx_t[:, j * F + f:j * F + f + 1], axis=0
                    ),
                )
            xp = pp_x.tile([96, 128], F32, tag="xp")
            nc.tensor.transpose(out=xp[:], in_=gb[:], identity=idn_t)
            nc.vector.tensor_copy(x_t[0:96, cols], xp[:])

            # MLP
            h1p = pp_h.tile([HID, 128], F32, tag="hp")
            nc.tensor.matmul(out=h1p[:], lhsT=w1s_t, rhs=x_t[:, cols],
                             start=True, stop=True)
            h1_t = hpool.tile([HID, 128], F32, tag="h")
            nc.scalar.activation(h1_t[:], h1p[:], RELU)
            h2p = pp_h.tile([HID, 128], F32, tag="hp")
            nc.tensor.matmul(out=h2p[:], lhsT=w2_t, rhs=h1_t[:],
                             start=True, stop=True)
            h2_t = hpool.tile([HID, 128], F32, tag="h")
            nc.scalar.activation(h2_t[:], h2p[:], RELU, bias=b2_t)

            # s / dense_emb rows
            sdp = pp_s.tile([20, 128], F32, tag="sd")
            nc.tensor.matmul(out=sdp[:], lhsT=sdw_t, rhs=x_t[:, cols],
                             start=True, stop=True)

            xsq = qpool.tile([96, 128], F32, tag="xsq")
            nc.vector.tensor_mul(xsq[:], x_t[0:96, cols], x_t[0:96, cols])
            sd2 = qpool.tile([20, 128], F32, tag="sd2")
            nc.scalar.activation(sd2[:], sdp[:], SQUARE)

            # final accumulation + sigmoid
            fin = pp_f.tile([1, 128], F32, tag="fin")
            nc.tensor.matmul(out=fin[:], lhsT=a1_t, rhs=x_t[:, cols],
                             start=True, stop=False)
            nc.tensor.matmul(out=fin[:], lhsT=esq_t, rhs=xsq[:],
                             start=False, stop=False)
            nc.tensor.matmul(out=fin[:], lhsT=es2_t, rhs=sd2[:],
                             start=False, stop=False)
            nc.tensor.matmul(out=fin[:], lhsT=w3_t, rhs=h2_t[:],
                             start=False, stop=True)
            if debug_dump:
                nc.vector.tensor_copy(fin_sb[:, cols], fin[:])
            nc.scalar.activation(out_sb[:, cols], fin[:], SIGMOID)

        nc.sync.dma_start(out_d[:], out_sb[:])
        if debug_dump:
            nc.sync.dma_start(xdmp_d[:], x_t[:])
            nc.sync.dma_start(fdmp_d[:], fin_sb[:])

    nc.compile()
    return nc


def _host_prep(sparse_feature, dense_feature, emb_table, W_dense, b_dense,
               w_first, b_first, W1, b1, W2, b2, W3, b3):
    """Build the augmented table, folded weights, and per-core in_maps."""
    f32 = np.float32
    emb_table = np.asarray(emb_table, dtype=f32)
    W_dense = np.asarray(W_dense, dtype=f32)      # [10, 7]
    b_dense = np.asarray(b_dense, dtype=f32)      # [10]
    w_first = np.asarray(w_first, dtype=f32)      # [S+7]
    b_first = np.asarray(b_first, dtype=f32)      # [1]
    W1 = np.asarray(W1, dtype=f32)                # [90, 128]
    b1 = np.asarray(b1, dtype=f32)                # [128]
    W2 = np.asarray(W2, dtype=f32)                # [128, 128]
    b2 = np.asarray(b2, dtype=f32)                # [128]
    W3 = np.asarray(W3, dtype=f32)                # [128, 1]
    b3 = np.asarray(b3, dtype=f32)                # [1]

    tab = np.zeros((S, RW), dtype=f32)
    tab[:, :EMB] = emb_table
    tab[:, EMB] = w_first[:S]

    w1s = np.zeros((KX, HID), dtype=f32)
    for f in range(F):
        w1s[f * RW:f * RW + EMB] = W1[f * EMB:(f + 1) * EMB]
    w1s[96:103] = W_dense.T @ W1[F * EMB:]               # [7,128]
    w1s[103] = b1 + b_dense @ W1[F * EMB:]

    sdw = np.zeros((KX, 20), dtype=f32)
    for f in range(F):
        for e in range(EMB):
            sdw[f * RW + e, e] = 1.0
    sdw[96:103, 0:10] = W_dense.T
    sdw[103, 0:10] = b_dense
    sdw[96:103, 10:20] = W_dense.T
    sdw[103, 10:20] = b_dense

    a1 = np.zeros((KX, 1), dtype=f32)
    for f in range(F):
        a1[f * RW + EMB] = 1.0
    a1[96:103, 0] = w_first[S:]
    a1[103] = b_first[0] + b3[0]

    esq = np.zeros((96, 1), dtype=f32)
    for f in range(F):
        esq[f * RW:f * RW + EMB] = -0.5
    es2 = np.zeros((20, 1), dtype=f32)
    es2[0:10] = 0.5
    es2[10:20] = -0.5

    idx_g = (np.asarray(sparse_feature, dtype=np.int64)
             + OFFSETS[None, :]).astype(np.int32)         # [B, F]
    dense = np.asarray(dense_feature, dtype=f32)          # [B, 7]

    wpk = np.zeros((128, 409), dtype=f32)
    wpk[:, 0:128] = np.eye(128, dtype=f32)
    wpk[0:KX, 128:256] = w1s
    wpk[:, 256:384] = W2
    wpk[:, 384] = b2
    wpk[0:KX, 385:405] = sdw
    wpk[0:KX, 405] = a1[:, 0]
    wpk[0:96, 406] = esq[:, 0]
    wpk[0:20, 407] = es2[:, 0]
    wpk[:, 408] = W3.reshape(HID)

    common = {"tab": tab, "wpk": wpk}
    in_maps = []
    for c in range(N_CORES):
        lo, hi = c * BL, (c + 1) * BL
        lg = idx_g[lo:hi].reshape(NBLK, 128, F)
        idxs = np.ascontiguousarray(
            lg.transpose(1, 0, 2).reshape(128, NBLK * F))  # [128, 128]
        dn8 = np.ones((8, BL), dtype=f32)
        dn8[:7] = dense[lo:hi].T
        in_maps.append(dict(common, idxs=idxs, dn8=dn8))
    return in_maps


def _get_program(debug_dump=False):
    key = ("nc", debug_dump)
    if key not in _cached:
        _cached[key] = _build_program(debug_dump)
    return _cached[key]


def run_on_device(in_maps, trace=False, debug_dump=False):
    """Run the SPMD program on 8 NeuronCores.  Returns (results, exec_time_ns)."""
    from concourse.bass_utils import run_bass_kernel_spmd

    nc = _get_program(debug_dump)
    res = run_bass_kernel_spmd(nc, in_maps, list(range(N_CORES)), trace=trace)
    return res.results, res.exec_time_ns


def kernel(**inputs):
    in_maps = _host_prep(**inputs)
    results, _ = run_on_device(in_maps, trace=False)
    out = np.concatenate([results[c]["out"].reshape(BL) for c in range(N_CORES)])
    return out.astype(np.float32)

